# revision 7
# baseline (speedup 1.0000x reference)
# Trainium2 Bass kernel for nn_AttentionBlock (GroupNorm -> QKV -> single-head
# attention over 64x64 tokens -> proj -> residual), B=4, C=256, H=W=64.
#
# Sharding: 8 cores = (batch b in 0..3) x (query-half in {0,1}).  Each core
# receives batch item b's full (C, N=4096) slab, rotated so that its own 2048
# query positions come first.  The program is identical on every core (pure
# SPMD, no collectives); the host slices inputs and reassembles the output.
#
# On-chip layout is channel-major (C on partitions) everywhere except V, which
# is produced directly token-major (n on partitions) so the P@V contraction
# needs no transposes.  Attention is computed as S^T (keys on partitions,
# queries on free axis): softmax reductions over keys become matmuls — the
# denominator l = sum_n exp(s) is one extra M=1 ones-row matmul accumulated
# alongside the P@V matmuls.  exp() skips max-subtraction: logits here are
# ~N(0,1) (max < ~7), far from fp32 overflow, and softmax is shift-invariant.
#
# All heavy matmuls run in bf16 with fp32 PSUM accumulation; GroupNorm stats
# and the softmax normalization stay fp32.

import numpy as np
import ml_dtypes

import concourse.bass as bass
import concourse.bacc as bacc
import concourse.mybir as mybir
import concourse.tile as tile
from concourse.bass_utils import run_bass_kernel_spmd

F32 = mybir.dt.float32
BF16 = mybir.dt.bfloat16

B = 4
C = 256
N = 4096          # tokens per batch item (64*64)
NH = 2048         # tokens per core (query half)
G = 32            # groups
GS = C // G       # channels per group
P = 128
CT = C // P       # 2 channel tiles
NT = N // P       # 32 key tiles
QB = NH // 512    # 4 query blocks of 512
EPS = 1e-6
LOGIT_SCALE = 1.0 / 16.0   # 1/sqrt(C)

TRACE = False
LAST_RESULT = None
_CACHED_NC = None


def _build_nc():
    nc = bacc.Bacc()

    x_in = nc.dram_tensor("x_in", [C, N], F32, kind="ExternalInput")
    wqkvT = nc.dram_tensor("wqkvT", [C, 3 * C], BF16, kind="ExternalInput")
    wprojT = nc.dram_tensor("wprojT", [C, C], BF16, kind="ExternalInput")
    bqkv = nc.dram_tensor("bqkv", [3 * C, 1], F32, kind="ExternalInput")
    bproj = nc.dram_tensor("bproj", [C, 1], F32, kind="ExternalInput")
    gamma_d = nc.dram_tensor("gamma", [C, 1], F32, kind="ExternalInput")
    beta_d = nc.dram_tensor("beta", [C, 1], F32, kind="ExternalInput")
    gsel_d = nc.dram_tensor("gsel", [C, G], F32, kind="ExternalInput")
    gbc_d = nc.dram_tensor("gbc", [G, C], F32, kind="ExternalInput")
    out_d = nc.dram_tensor("out", [C, NH], F32, kind="ExternalOutput")

    with tile.TileContext(nc) as tc:
        with (
            tc.tile_pool(name="persist", bufs=1) as pp,
            tc.tile_pool(name="small", bufs=1) as sp,
            tc.tile_pool(name="ptiles", bufs=3) as ptp,
            tc.tile_pool(name="work", bufs=2) as wkp,
        ):
            # ---- load inputs -------------------------------------------------
            x_t = []
            for i in range(CT):
                xt = pp.tile([P, N], F32, tag=f"x{i}")
                nc.sync.dma_start(out=xt, in_=x_in[i * P:(i + 1) * P, :])
                x_t.append(xt)

            wq_t = []
            for i in range(CT):
                wt = pp.tile([P, 3 * C], BF16, tag=f"wqkv{i}")
                nc.sync.dma_start(out=wt, in_=wqkvT[i * P:(i + 1) * P, :])
                wq_t.append(wt)
            wp_t = []
            for i in range(CT):
                wt = pp.tile([P, C], BF16, tag=f"wproj{i}")
                nc.sync.dma_start(out=wt, in_=wprojT[i * P:(i + 1) * P, :])
                wp_t.append(wt)

            # (768,1) biases -> (128, 6): column j holds rows [128j, 128j+128)
            bq_sb = sp.tile([P, 6], F32, tag="bqkv")
            nc.sync.dma_start(
                out=bq_sb,
                in_=bass.AP(tensor=bqkv, offset=0, ap=[[1, P], [P, 6]]),
            )
            bpj_sb = sp.tile([P, CT], F32, tag="bproj")
            nc.sync.dma_start(
                out=bpj_sb,
                in_=bass.AP(tensor=bproj, offset=0, ap=[[1, P], [P, CT]]),
            )
            gam_sb = sp.tile([P, CT], F32, tag="gamma")
            nc.sync.dma_start(
                out=gam_sb,
                in_=bass.AP(tensor=gamma_d, offset=0, ap=[[1, P], [P, CT]]),
            )
            bet_sb = sp.tile([P, CT], F32, tag="beta")
            nc.sync.dma_start(
                out=bet_sb,
                in_=bass.AP(tensor=beta_d, offset=0, ap=[[1, P], [P, CT]]),
            )
            # v-bias broadcast to all partitions (v is token-major)
            bv_bc = sp.tile([P, C], F32, tag="bvbc")
            nc.sync.dma_start(
                out=bv_bc,
                in_=bass.AP(tensor=bqkv, offset=2 * C, ap=[[0, P], [1, C]]),
            )
            # fp32 matmuls lower to a single instruction with one sync-wait
            # slot, so their operands must all come from one engine: launder
            # the DMA-loaded selector matrices through a DVE copy.
            gsel_t = []
            for i in range(CT):
                gt0 = sp.tile([P, G], F32, tag=f"gseld{i}", name=f"gt0_{i}")
                nc.sync.dma_start(out=gt0, in_=gsel_d[i * P:(i + 1) * P, :])
                gt = sp.tile([P, G], F32, tag=f"gsel{i}", name=f"gt_{i}")
                nc.vector.tensor_copy(gt, gt0)
                gsel_t.append(gt)
            gbc0 = sp.tile([G, C], F32, tag="gbcd")
            nc.sync.dma_start(out=gbc0, in_=gbc_d[:, :])
            gbc_sb = sp.tile([G, C], F32, tag="gbc")
            nc.vector.tensor_copy(gbc_sb, gbc0)

            ones_l = sp.tile([P, 1], BF16, tag="ones_l")
            nc.vector.memset(ones_l, 1.0)
            eps_t = sp.tile([G, 1], F32, tag="eps")
            nc.vector.memset(eps_t, EPS)

            # ---- GroupNorm statistics ---------------------------------------
            # per-channel mean/var via bn_stats (8 subgroups of 512)
            with tc.tile_pool(name="gn_ps", bufs=1, space="PSUM") as gnps:
                stat2 = []
                for i in range(CT):
                    bst = sp.tile([P, 8, 6], F32, tag=f"bnst{i}")
                    for s in range(8):
                        nc.vector.bn_stats(
                            out=bst[:, s, :],
                            in_=x_t[i][:, s * 512:(s + 1) * 512],
                        )
                    mv = sp.tile([P, 2], F32, tag=f"mv{i}")
                    nc.vector.bn_aggr(out=mv, in_=bst)
                    st = sp.tile([P, 2], F32, tag=f"stat2{i}")
                    nc.vector.tensor_copy(st[:, 0:1], mv[:, 0:1])
                    # m2 = var + mean^2
                    nc.vector.tensor_mul(st[:, 1:2], mv[:, 0:1], mv[:, 0:1])
                    nc.vector.tensor_add(st[:, 1:2], st[:, 1:2], mv[:, 1:2])
                    stat2.append(st)

                # group aggregate: (32, 2) = sum_c gsel[c,g]/8 * [mean_c, m2_c]
                ps_g = gnps.tile([G, 2], F32, tag="psg")
                nc.tensor.matmul(ps_g, gsel_t[0], stat2[0], start=True, stop=False)
                nc.tensor.matmul(ps_g, gsel_t[1], stat2[1], start=False, stop=True)

                grp = sp.tile([G, 2], F32, tag="grp")
                nc.vector.tensor_copy(grp, ps_g)
                # var_g = m2_g - mean_g^2 ; rstd = 1/sqrt(var+eps)
                vtmp = sp.tile([G, 1], F32, tag="vtmp")
                nc.vector.tensor_mul(vtmp, grp[:, 0:1], grp[:, 0:1])
                nc.vector.tensor_sub(vtmp, grp[:, 1:2], vtmp)
                srt = sp.tile([G, 1], F32, tag="srt")
                nc.scalar.activation(
                    out=srt, in_=vtmp,
                    func=mybir.ActivationFunctionType.Sqrt,
                    bias=eps_t, scale=1.0,
                )
                mr_g = sp.tile([G, 2], F32, tag="mrg")
                nc.vector.tensor_copy(mr_g[:, 0:1], grp[:, 0:1])
                nc.vector.reciprocal(mr_g[:, 1:2], srt)

                # broadcast back to channels: (128, 2) per c-tile
                scale_c, shift_c = [], []
                for i in range(CT):
                    ps_c = gnps.tile([P, 2], F32, tag="psc", bufs=2)
                    nc.tensor.matmul(
                        ps_c, gbc_sb[:, i * P:(i + 1) * P], mr_g,
                        start=True, stop=True,
                    )
                    sc = sp.tile([P, 1], F32, tag=f"scale{i}")
                    sh = sp.tile([P, 1], F32, tag=f"shift{i}")
                    # scale = rstd * gamma ; shift = beta - mean * scale
                    nc.vector.tensor_mul(sc, ps_c[:, 1:2], gam_sb[:, i:i + 1])
                    nc.vector.tensor_mul(sh, ps_c[:, 0:1], sc)
                    nc.vector.tensor_sub(sh, bet_sb[:, i:i + 1], sh)
                    scale_c.append(sc)
                    shift_c.append(sh)

            # ---- h = GroupNorm(x) in bf16 (ACT); x += bproj in-place (DVE) --
            h_t = []
            for i in range(CT):
                ht = pp.tile([P, N], BF16, tag=f"h{i}")
                nc.scalar.activation(
                    out=ht, in_=x_t[i],
                    func=mybir.ActivationFunctionType.Identity,
                    bias=shift_c[i], scale=scale_c[i],
                )
                h_t.append(ht)
            for i in range(CT):
                # x (residual half) + bproj, in place
                nc.vector.tensor_scalar_add(
                    out=x_t[i][:, 0:NH], in0=x_t[i][:, 0:NH],
                    scalar1=bpj_sb[:, i:i + 1],
                )

            # ---- QKV ---------------------------------------------------------
            q_t = [pp.tile([P, NH], BF16, tag=f"q{i}", name=f"q{i}") for i in range(CT)]
            k_t = [pp.tile([P, N], BF16, tag=f"k{i}", name=f"k{i}") for i in range(CT)]
            v_sb = pp.tile([P, NT, C], BF16, tag="v")

            with tc.tile_pool(name="qkv_ps", bufs=1, space="PSUM") as qps:
                for co in range(CT):   # q: only our half
                    for nb in range(QB):
                        ps = qps.tile([P, 512], F32, tag="qk", bufs=3)
                        for ci in range(CT):
                            nc.tensor.matmul(
                                ps,
                                wq_t[ci][:, co * P:(co + 1) * P],
                                h_t[ci][:, nb * 512:(nb + 1) * 512],
                                start=(ci == 0), stop=(ci == CT - 1),
                            )
                        nc.scalar.activation(
                            out=q_t[co][:, nb * 512:(nb + 1) * 512], in_=ps,
                            func=mybir.ActivationFunctionType.Identity,
                            bias=bq_sb[:, co:co + 1], scale=1.0,
                        )
                for co in range(CT):   # k: full token range
                    for nb in range(N // 512):
                        ps = qps.tile([P, 512], F32, tag="qk", bufs=3)
                        for ci in range(CT):
                            nc.tensor.matmul(
                                ps,
                                wq_t[ci][:, C + co * P:C + (co + 1) * P],
                                h_t[ci][:, nb * 512:(nb + 1) * 512],
                                start=(ci == 0), stop=(ci == CT - 1),
                            )
                        nc.scalar.activation(
                            out=k_t[co][:, nb * 512:(nb + 1) * 512], in_=ps,
                            func=mybir.ActivationFunctionType.Identity,
                            bias=bq_sb[:, 2 + co:3 + co], scale=1.0,
                        )
                for i in range(NT):    # v: token-major
                    ps = qps.tile([P, C], F32, tag="v", bufs=3)
                    for ci in range(CT):
                        nc.tensor.matmul(
                            ps,
                            h_t[ci][:, i * P:(i + 1) * P],
                            wq_t[ci][:, 2 * C:3 * C],
                            start=(ci == 0), stop=(ci == CT - 1),
                        )
                    nc.vector.tensor_add(v_sb[:, i, :], ps, bv_bc)

            # ---- attention + proj + residual, per query block ----------------
            with tc.tile_pool(name="att_ps", bufs=1, space="PSUM") as aps:
                for qb in range(QB):
                    qsl = slice(qb * 512, (qb + 1) * 512)
                    o0 = aps.tile([P, 512], F32, tag="o0")
                    o1 = aps.tile([P, 512], F32, tag="o1")
                    lps = aps.tile([1, 512], F32, tag="l")

                    # software-pipelined by one: S(i+1) is emitted before PV(i)
                    def s_mms(i):
                        s = aps.tile([P, 512], F32, tag="s", bufs=3)
                        for ci in range(CT):
                            nc.tensor.matmul(
                                s,
                                k_t[ci][:, i * P:(i + 1) * P],
                                q_t[ci][:, qsl],
                                start=(ci == 0), stop=(ci == CT - 1),
                            )
                        return s

                    s_cur = s_mms(0)
                    for i in range(NT):
                        p = ptp.tile([P, 512], BF16, tag="p")
                        nc.scalar.activation(
                            out=p, in_=s_cur,
                            func=mybir.ActivationFunctionType.Exp,
                            bias=0.0, scale=LOGIT_SCALE,
                        )
                        if i + 1 < NT:
                            s_cur = s_mms(i + 1)
                        nc.tensor.matmul(
                            o0, v_sb[:, i, 0:P], p,
                            start=(i == 0), stop=(i == NT - 1),
                        )
                        nc.tensor.matmul(
                            o1, v_sb[:, i, P:C], p,
                            start=(i == 0), stop=(i == NT - 1),
                        )
                        nc.tensor.matmul(
                            lps, ones_l, p,
                            start=(i == 0), stop=(i == NT - 1),
                        )

                    recip = wkp.tile([1, 512], F32, tag="recip")
                    nc.vector.reciprocal(recip, lps)
                    rbc = wkp.tile([P, 512], F32, tag="rbc")
                    nc.gpsimd.partition_broadcast(rbc, recip)

                    o_sb = []
                    for j, ops_ in enumerate((o0, o1)):
                        ob = wkp.tile([P, 512], BF16, tag=f"osb{j}")
                        nc.vector.tensor_copy(ob, ops_)
                        o_sb.append(ob)

                    for co in range(CT):
                        pj = aps.tile([P, 512], F32, tag=f"pj{co}")
                        for ci in range(CT):
                            nc.tensor.matmul(
                                pj,
                                wp_t[ci][:, co * P:(co + 1) * P],
                                o_sb[ci],
                                start=(ci == 0), stop=(ci == CT - 1),
                            )
                        f = wkp.tile([P, 512], F32, tag=f"f{co}")
                        nc.vector.tensor_mul(f, pj, rbc)
                        nc.vector.tensor_add(f, f, x_t[co][:, qsl])
                        nc.sync.dma_start(
                            out=out_d[co * P:(co + 1) * P, qsl], in_=f
                        )
    nc.finalize()
    return nc


def _host_inputs(x, gamma, beta, w_qkv, b_qkv, w_proj, b_proj):
    x4 = np.ascontiguousarray(np.asarray(x, np.float32).reshape(B, C, N))
    wqkvT = np.ascontiguousarray(
        np.asarray(w_qkv, np.float32).T).astype(ml_dtypes.bfloat16)
    wprojT = np.ascontiguousarray(
        np.asarray(w_proj, np.float32).T).astype(ml_dtypes.bfloat16)
    bqkv = np.ascontiguousarray(np.asarray(b_qkv, np.float32).reshape(3 * C, 1))
    bproj = np.ascontiguousarray(np.asarray(b_proj, np.float32).reshape(C, 1))
    gam = np.ascontiguousarray(np.asarray(gamma, np.float32).reshape(C, 1))
    bet = np.ascontiguousarray(np.asarray(beta, np.float32).reshape(C, 1))

    # bn_aggr gives per-channel mean/var over the N positions, so the group
    # combine only averages the GS channels in each group: weight 1/GS.
    gsel = np.zeros((C, G), np.float32)
    gbc = np.zeros((G, C), np.float32)
    for c in range(C):
        gsel[c, c // GS] = 1.0 / GS
        gbc[c // GS, c] = 1.0

    shared = dict(wqkvT=wqkvT, wprojT=wprojT, bqkv=bqkv, bproj=bproj,
                  gamma=gam, beta=bet, gsel=gsel, gbc=gbc)
    in_maps = []
    for core in range(8):
        b, half = divmod(core, 2)
        xs = x4[b]
        if half:
            xs = np.concatenate([xs[:, NH:], xs[:, :NH]], axis=1)
        in_maps.append(dict(x_in=np.ascontiguousarray(xs), **shared))
    return in_maps


def kernel(x, gamma, beta, w_qkv, b_qkv, w_proj, b_proj):
    global _CACHED_NC, LAST_RESULT
    if _CACHED_NC is None:
        _CACHED_NC = _build_nc()
    in_maps = _host_inputs(x, gamma, beta, w_qkv, b_qkv, w_proj, b_proj)
    res = run_bass_kernel_spmd(
        _CACHED_NC, in_maps, core_ids=list(range(8)), trace=TRACE
    )
    LAST_RESULT = res
    out = np.empty((B, C, N), np.float32)
    for core in range(8):
        b, half = divmod(core, 2)
        out[b][:, half * NH:(half + 1) * NH] = res.results[core]["out"]
    return out.reshape(B, C, 64, 64)


# revision 20
# speedup vs baseline: 440612.9817x; 440612.9817x over previous
# Trainium2 Bass kernel for nn_AttentionBlock (GroupNorm -> QKV -> single-head
# attention over 64x64 tokens -> proj -> residual), B=4, C=256, H=W=64.
#
# Sharding: 8 cores = (batch b in 0..3) x (query-half in {0,1}).  Each core
# receives batch item b's full (C, N=4096) slab, rotated so that its own 2048
# query positions come first.  The program is identical on every core (pure
# SPMD, no collectives); the host slices inputs and reassembles the output.
#
# On-chip layout is channel-major (C on partitions) everywhere except V, which
# is produced directly token-major (n on partitions) so the P@V contraction
# needs no transposes.  Attention is computed as S^T (keys on partitions,
# queries on free axis).  exp() skips max-subtraction: logits here are ~N(0,1)
# (max < ~7), far from fp32 overflow, and softmax is shift-invariant.
#
# The softmax denominator l[q] = sum_n exp(s[n,q]) is a cross-partition sum:
# the 32 exp'd key-tiles are accumulated elementwise on the (otherwise idle)
# GpSimd and Vector engines into two (128,512) partials, and a single fp32
# ones-vector matmul folds the 128 partitions into l.  This keeps the
# TensorEngine (the bottleneck) free of the 128 M=1 matmuls it would otherwise
# spend ~27us on.
#
# All heavy matmuls run in bf16 with fp32 PSUM accumulation; GroupNorm stats
# and the softmax normalization stay fp32.  S-psum / P tiles / q,k drains are
# processed as (128,1024) two-bank tensor ops to halve instruction overhead.

import contextlib

import numpy as np
import ml_dtypes

import concourse.bass as bass
import concourse.bacc as bacc
import concourse.mybir as mybir
import concourse.tile as tile
from concourse.bass_utils import run_bass_kernel_spmd

F32 = mybir.dt.float32
BF16 = mybir.dt.bfloat16

B = 4
C = 256
N = 4096          # tokens per batch item (64*64)
NH = 2048         # tokens per core (query half)
G = 32            # groups
GS = C // G       # channels per group
P = 128
CT = C // P       # 2 channel tiles
NT = N // P       # 32 key tiles
QB = NH // 512    # 4 query blocks of 512
EPS = 1e-6
LOGIT_SCALE = 1.0 / 16.0   # 1/sqrt(C)

TRACE = False
PHASES = ("gn", "qkv", "attn")
LAST_RESULT = None
_CACHED_NC = None


def _build_nc(loop_k=None):
    nc = bacc.Bacc()

    x_in = nc.dram_tensor("x_in", [C, N], F32, kind="ExternalInput")
    wqkvT = nc.dram_tensor("wqkvT", [C, 3 * C], BF16, kind="ExternalInput")
    wprojT = nc.dram_tensor("wprojT", [C, C], BF16, kind="ExternalInput")
    bqkv = nc.dram_tensor("bqkv", [3 * C, 1], F32, kind="ExternalInput")
    bproj = nc.dram_tensor("bproj", [C, 1], F32, kind="ExternalInput")
    gamma_d = nc.dram_tensor("gamma", [C, 1], F32, kind="ExternalInput")
    beta_d = nc.dram_tensor("beta", [C, 1], F32, kind="ExternalInput")
    gsel_d = nc.dram_tensor("gsel", [C, G], F32, kind="ExternalInput")
    gbc_d = nc.dram_tensor("gbc", [G, C], F32, kind="ExternalInput")
    out_d = nc.dram_tensor("out", [C, NH], F32, kind="ExternalOutput")

    with tile.TileContext(nc) as tc:
        with (
            tc.tile_pool(name="persist", bufs=1) as pp,
            tc.tile_pool(name="small", bufs=1) as sp,
            tc.tile_pool(name="ptiles", bufs=4) as ptp,
            tc.tile_pool(name="work", bufs=2) as wkp,
            tc.For_i(0, loop_k, 1) if loop_k else contextlib.nullcontext(),
        ):
            # ---- load inputs -------------------------------------------------
            x_t = []
            for i in range(CT):
                xt = pp.tile([P, N], F32, tag=f"x{i}", name=f"x{i}")
                # split the load so bn_stats can start on early chunks
                for ch in range(4):
                    nc.sync.dma_start(
                        out=xt[:, ch * (N // 4):(ch + 1) * (N // 4)],
                        in_=x_in[i * P:(i + 1) * P,
                                 ch * (N // 4):(ch + 1) * (N // 4)])
                x_t.append(xt)

            wq_t = []
            for i in range(CT):
                wt = pp.tile([P, 3 * C], BF16, tag=f"wqkv{i}", name=f"wq{i}")
                nc.sync.dma_start(out=wt, in_=wqkvT[i * P:(i + 1) * P, :])
                wq_t.append(wt)
            wp_t = []
            for i in range(CT):
                wt = pp.tile([P, C], BF16, tag=f"wproj{i}", name=f"wp{i}")
                nc.sync.dma_start(out=wt, in_=wprojT[i * P:(i + 1) * P, :])
                wp_t.append(wt)

            # (768,1) biases -> (128, 6): column j holds rows [128j, 128j+128)
            bq_sb = sp.tile([P, 6], F32, tag="bqkv")
            nc.sync.dma_start(
                out=bq_sb,
                in_=bass.AP(tensor=bqkv, offset=0, ap=[[1, P], [P, 6]]),
            )
            bpj_sb = sp.tile([P, CT], F32, tag="bproj")
            nc.sync.dma_start(
                out=bpj_sb,
                in_=bass.AP(tensor=bproj, offset=0, ap=[[1, P], [P, CT]]),
            )
            gam_sb = sp.tile([P, CT], F32, tag="gamma")
            nc.sync.dma_start(
                out=gam_sb,
                in_=bass.AP(tensor=gamma_d, offset=0, ap=[[1, P], [P, CT]]),
            )
            bet_sb = sp.tile([P, CT], F32, tag="beta")
            nc.sync.dma_start(
                out=bet_sb,
                in_=bass.AP(tensor=beta_d, offset=0, ap=[[1, P], [P, CT]]),
            )
            # fp32 matmuls lower to a single instruction with one sync-wait
            # slot, so their operands must all come from one engine: launder
            # the DMA-loaded selector matrices through a DVE copy.
            gsel_t = []
            for i in range(CT):
                gt0 = sp.tile([P, G], F32, tag=f"gseld{i}", name=f"gt0_{i}")
                nc.sync.dma_start(out=gt0, in_=gsel_d[i * P:(i + 1) * P, :])
                gt = sp.tile([P, G], F32, tag=f"gsel{i}", name=f"gt_{i}")
                nc.vector.tensor_copy(gt, gt0)
                gsel_t.append(gt)
            gbc0 = sp.tile([G, C], F32, tag="gbcd")
            nc.sync.dma_start(out=gbc0, in_=gbc_d[:, :])
            gbc_sb = sp.tile([G, C], F32, tag="gbc")
            nc.vector.tensor_copy(gbc_sb, gbc0)

            ones_f = sp.tile([P, 1], F32, tag="ones_f")
            nc.vector.memset(ones_f, 1.0)
            eps_t = sp.tile([G, 1], F32, tag="eps")
            nc.vector.memset(eps_t, EPS)

            # ---- GroupNorm statistics ---------------------------------------
            # per-channel mean/var via bn_stats (8 subgroups of 512)
            with tc.tile_pool(name="gn_ps", bufs=1, space="PSUM") as gnps:
                stat2 = []
                for i in range(CT):
                    bst = sp.tile([P, 8, 6], F32, tag=f"bnst{i}", name=f"bnst{i}")
                    for s in range(8):
                        nc.vector.bn_stats(
                            out=bst[:, s, :],
                            in_=x_t[i][:, s * 512:(s + 1) * 512],
                        )
                    mv = sp.tile([P, 2], F32, tag=f"mv{i}", name=f"mv{i}")
                    nc.vector.bn_aggr(out=mv, in_=bst)
                    st = sp.tile([P, 2], F32, tag=f"stat2{i}", name=f"st{i}")
                    nc.vector.tensor_copy(st[:, 0:1], mv[:, 0:1])
                    # m2 = var + mean^2
                    nc.vector.tensor_mul(st[:, 1:2], mv[:, 0:1], mv[:, 0:1])
                    nc.vector.tensor_add(st[:, 1:2], st[:, 1:2], mv[:, 1:2])
                    stat2.append(st)

                # group aggregate: (32, 2) = sum_c gsel[c,g]/8 * [mean_c, m2_c]
                ps_g = gnps.tile([G, 2], F32, tag="psg")
                nc.tensor.matmul(ps_g, gsel_t[0], stat2[0], start=True, stop=False)
                nc.tensor.matmul(ps_g, gsel_t[1], stat2[1], start=False, stop=True)

                grp = sp.tile([G, 2], F32, tag="grp")
                nc.vector.tensor_copy(grp, ps_g)
                # var_g = m2_g - mean_g^2 ; rstd = 1/sqrt(var+eps)
                vtmp = sp.tile([G, 1], F32, tag="vtmp")
                nc.vector.tensor_mul(vtmp, grp[:, 0:1], grp[:, 0:1])
                nc.vector.tensor_sub(vtmp, grp[:, 1:2], vtmp)
                srt = sp.tile([G, 1], F32, tag="srt")
                nc.scalar.activation(
                    out=srt, in_=vtmp,
                    func=mybir.ActivationFunctionType.Sqrt,
                    bias=eps_t, scale=1.0,
                )
                mr_g = sp.tile([G, 2], F32, tag="mrg")
                nc.vector.tensor_copy(mr_g[:, 0:1], grp[:, 0:1])
                nc.vector.reciprocal(mr_g[:, 1:2], srt)

                # broadcast back to channels: (128, 2) per c-tile
                scale_c, shift_c = [], []
                for i in range(CT):
                    ps_c = gnps.tile([P, 2], F32, tag="psc", bufs=2, name=f"psc{i}")
                    nc.tensor.matmul(
                        ps_c, gbc_sb[:, i * P:(i + 1) * P], mr_g,
                        start=True, stop=True,
                    )
                    sc = sp.tile([P, 1], F32, tag=f"scale{i}", name=f"sc{i}")
                    sh = sp.tile([P, 1], F32, tag=f"shift{i}", name=f"sh{i}")
                    # scale = rstd * gamma ; shift = beta - mean * scale
                    nc.vector.tensor_mul(sc, ps_c[:, 1:2], gam_sb[:, i:i + 1])
                    nc.vector.tensor_mul(sh, ps_c[:, 0:1], sc)
                    nc.vector.tensor_sub(sh, bet_sb[:, i:i + 1], sh)
                    scale_c.append(sc)
                    shift_c.append(sh)

            # ---- h = GroupNorm(x) in bf16 (ACT); x += bproj in-place (DVE) --
            h_t = []
            for i in range(CT):
                ht = pp.tile([P, N], BF16, tag=f"h{i}", name=f"h{i}")
                if i == 0:
                    nc.scalar.activation(
                        out=ht, in_=x_t[i],
                        func=mybir.ActivationFunctionType.Identity,
                        bias=shift_c[i], scale=scale_c[i],
                    )
                else:
                    nc.vector.tensor_scalar(
                        out=ht, in0=x_t[i],
                        scalar1=scale_c[i], scalar2=shift_c[i],
                        op0=mybir.AluOpType.mult, op1=mybir.AluOpType.add,
                    )
                h_t.append(ht)
            for i in range(CT):
                # x (residual half) + bproj, in place
                nc.vector.tensor_scalar_add(
                    out=x_t[i][:, 0:NH], in0=x_t[i][:, 0:NH],
                    scalar1=bpj_sb[:, i:i + 1],
                )

            # ---- QKV ---------------------------------------------------------
            q_t = [pp.tile([P, NH], BF16, tag=f"q{i}", name=f"q{i}")
                   for i in range(CT)]
            k_t = [pp.tile([P, N], BF16, tag=f"k{i}", name=f"k{i}")
                   for i in range(CT)]
            v_sb = pp.tile([P, NT, C], BF16, tag="v")

            if "qkv" not in PHASES:
                qps = None
            else:
              with tc.tile_pool(name="qkv_ps", bufs=1, space="PSUM") as qps:
                for co in range(CT):   # q: only our half, 1024-wide blocks
                    for nb in range(NH // 1024):
                        ps = qps.tile([P, 1024], F32, tag="qk", bufs=2, name="psq")
                        for r in range(2):   # psum bank per matmul group
                            for ci in range(CT):
                                nc.tensor.matmul(
                                    ps[:, r * 512:(r + 1) * 512],
                                    wq_t[ci][:, co * P:(co + 1) * P],
                                    h_t[ci][:, nb * 1024 + r * 512:
                                            nb * 1024 + (r + 1) * 512],
                                    start=(ci == 0), stop=(ci == CT - 1),
                                )
                        if (co + nb) % 2 == 0:
                            nc.scalar.activation(
                                out=q_t[co][:, nb * 1024:(nb + 1) * 1024],
                                in_=ps,
                                func=mybir.ActivationFunctionType.Identity,
                                bias=bq_sb[:, co:co + 1], scale=1.0,
                            )
                        else:
                            nc.vector.tensor_scalar_add(
                                out=q_t[co][:, nb * 1024:(nb + 1) * 1024],
                                in0=ps, scalar1=bq_sb[:, co:co + 1],
                            )
                for co in range(CT):   # k: full token range
                    for nb in range(N // 1024):
                        ps = qps.tile([P, 1024], F32, tag="qk", bufs=2, name="psk")
                        for r in range(2):   # psum bank per matmul group
                            for ci in range(CT):
                                nc.tensor.matmul(
                                    ps[:, r * 512:(r + 1) * 512],
                                    wq_t[ci][:, C + co * P:C + (co + 1) * P],
                                    h_t[ci][:, nb * 1024 + r * 512:
                                            nb * 1024 + (r + 1) * 512],
                                    start=(ci == 0), stop=(ci == CT - 1),
                                )
                        if (co + nb) % 2 == 0:
                            nc.scalar.activation(
                                out=k_t[co][:, nb * 1024:(nb + 1) * 1024],
                                in_=ps,
                                func=mybir.ActivationFunctionType.Identity,
                                bias=bq_sb[:, 2 + co:3 + co], scale=1.0,
                            )
                        else:
                            nc.vector.tensor_scalar_add(
                                out=k_t[co][:, nb * 1024:(nb + 1) * 1024],
                                in0=ps, scalar1=bq_sb[:, 2 + co:3 + co],
                            )
                for i2 in range(NT // 2):   # v: token-major, paired tiles
                    ps = qps.tile([P, 2, C], F32, tag="v", bufs=3, name="psv")
                    for r in range(2):
                        i = 2 * i2 + r
                        for ci in range(CT):
                            nc.tensor.matmul(
                                ps[:, r, :],
                                h_t[ci][:, i * P:(i + 1) * P],
                                wq_t[ci][:, 2 * C:3 * C],
                                start=(ci == 0), stop=(ci == CT - 1),
                            )
                    # v bias is folded into bproj on the host
                    if i2 % 2 == 0:
                        nc.scalar.activation(
                            out=v_sb[:, 2 * i2:2 * i2 + 2, :], in_=ps,
                            func=mybir.ActivationFunctionType.Copy,
                        )
                    else:
                        nc.vector.tensor_copy(
                            v_sb[:, 2 * i2:2 * i2 + 2, :], ps)

            # ---- attention + proj + residual, per query block ----------------
            with tc.tile_pool(name="att_ps", bufs=1, space="PSUM") as aps:
                for qb in range(QB):
                    qsl = slice(qb * 512, (qb + 1) * 512)
                    o01 = aps.tile([P, 2, 512], F32, tag="o01", name="o01")
                    # two elementwise accumulators for l, on Pool and DVE
                    lac = []
                    for e, eng in ((0, nc.gpsimd), (1, nc.vector)):
                        la = wkp.tile([P, 512], F32, tag=f"lac{e}", name=f"lac{e}")
                        eng.memset(la, 0.0)
                        lac.append(la)

                    # pairs of key tiles; S-mms fill a 2-bank psum, one exp
                    def s_mms(i2):
                        s = aps.tile([P, 2, 512], F32, tag="s", bufs=2,
                                     name="s2")
                        for r in range(2):
                            i = 2 * i2 + r
                            for ci in range(CT):
                                nc.tensor.matmul(
                                    s[:, r, :],
                                    k_t[ci][:, i * P:(i + 1) * P],
                                    q_t[ci][:, qsl],
                                    start=(ci == 0), stop=(ci == CT - 1),
                                )
                        return s

                    s_cur = s_mms(0)
                    for i2 in range(NT // 2):
                        p2 = ptp.tile([P, 2, 512], BF16, tag="p", name="p2")
                        nc.scalar.activation(
                            out=p2, in_=s_cur,
                            func=mybir.ActivationFunctionType.Exp,
                            bias=0.0, scale=LOGIT_SCALE,
                        )
                        if i2 + 1 < NT // 2:
                            s_cur = s_mms(i2 + 1)
                        for r in range(2):
                            i = 2 * i2 + r
                            nc.tensor.matmul(
                                o01[:, 0, :], v_sb[:, i, 0:P], p2[:, r, :],
                                start=(i == 0), stop=(i == NT - 1),
                            )
                            nc.tensor.matmul(
                                o01[:, 1, :], v_sb[:, i, P:C], p2[:, r, :],
                                start=(i == 0), stop=(i == NT - 1),
                            )
                        # l partials: elementwise adds on Pool / DVE
                        nc.gpsimd.tensor_add(lac[0], lac[0], p2[:, 0, :])
                        nc.vector.tensor_add(lac[1], lac[1], p2[:, 1, :])

                    # fold partitions: l = ones.T @ (lac0 + lac1), then 1/l
                    lps = aps.tile([1, 512], F32, tag="l", name="lps")
                    nc.vector.tensor_add(lac[1], lac[1], lac[0])
                    nc.tensor.matmul(lps, ones_f, lac[1], start=True, stop=True)
                    recip = wkp.tile([1, 512], F32, tag="recip", name="recip")
                    nc.vector.reciprocal(recip, lps)
                    rbc = wkp.tile([P, 512], F32, tag="rbc", name="rbc")
                    nc.gpsimd.partition_broadcast(rbc, recip)

                    o_sb = wkp.tile([P, 2, 512], BF16, tag="osb", name="osb")
                    nc.scalar.activation(
                        out=o_sb, in_=o01,
                        func=mybir.ActivationFunctionType.Copy)

                    pj = aps.tile([P, 2, 512], F32, tag="s", bufs=2, name="pj")
                    for co in range(CT):
                        for ci in range(CT):
                            nc.tensor.matmul(
                                pj[:, co, :],
                                wp_t[ci][:, co * P:(co + 1) * P],
                                o_sb[:, ci, :],
                                start=(ci == 0), stop=(ci == CT - 1),
                            )
                    for co in range(CT):
                        f = wkp.tile([P, 512], F32, tag=f"f{co}", name=f"f{co}")
                        nc.vector.tensor_mul(f, pj[:, co, :], rbc)
                        nc.vector.tensor_add(f, f, x_t[co][:, qsl])
                        nc.sync.dma_start(
                            out=out_d[co * P:(co + 1) * P, qsl], in_=f
                        )
    nc.finalize()
    return nc


def _host_inputs(x, gamma, beta, w_qkv, b_qkv, w_proj, b_proj):
    x4 = np.ascontiguousarray(np.asarray(x, np.float32).reshape(B, C, N))
    wqkvT = np.ascontiguousarray(
        np.asarray(w_qkv, np.float32).T).astype(ml_dtypes.bfloat16)
    wprojT = np.ascontiguousarray(
        np.asarray(w_proj, np.float32).T).astype(ml_dtypes.bfloat16)
    bqkv = np.ascontiguousarray(np.asarray(b_qkv, np.float32).reshape(3 * C, 1))
    # v-bias is applied on the host side of the algebra:
    # P@(V+b_v)/l = (P@V)/l + b_v, so proj(..)+b_proj gains w_proj @ b_v.
    bproj_eff = (np.asarray(b_proj, np.float32)
                 + np.asarray(w_proj, np.float32) @ np.asarray(
                     b_qkv, np.float32)[2 * C:3 * C])
    bproj = np.ascontiguousarray(bproj_eff.reshape(C, 1))
    gam = np.ascontiguousarray(np.asarray(gamma, np.float32).reshape(C, 1))
    bet = np.ascontiguousarray(np.asarray(beta, np.float32).reshape(C, 1))

    # bn_aggr gives per-channel mean/var over the N positions, so the group
    # combine only averages the GS channels in each group: weight 1/GS.
    gsel = np.zeros((C, G), np.float32)
    gbc = np.zeros((G, C), np.float32)
    for c in range(C):
        gsel[c, c // GS] = 1.0 / GS
        gbc[c // GS, c] = 1.0

    shared = dict(wqkvT=wqkvT, wprojT=wprojT, bqkv=bqkv, bproj=bproj,
                  gamma=gam, beta=bet, gsel=gsel, gbc=gbc)
    in_maps = []
    for core in range(8):
        b, half = divmod(core, 2)
        xs = x4[b]
        if half:
            xs = np.concatenate([xs[:, NH:], xs[:, :NH]], axis=1)
        in_maps.append(dict(x_in=np.ascontiguousarray(xs), **shared))
    return in_maps


def kernel(x, gamma, beta, w_qkv, b_qkv, w_proj, b_proj):
    global _CACHED_NC, LAST_RESULT
    if _CACHED_NC is None:
        _CACHED_NC = _build_nc()
    in_maps = _host_inputs(x, gamma, beta, w_qkv, b_qkv, w_proj, b_proj)
    res = run_bass_kernel_spmd(
        _CACHED_NC, in_maps, core_ids=list(range(8)), trace=TRACE
    )
    LAST_RESULT = res
    out = np.empty((B, C, N), np.float32)
    for core in range(8):
        b, half = divmod(core, 2)
        out[b][:, half * NH:(half + 1) * NH] = res.results[core]["out"]
    return out.reshape(B, C, 64, 64)


# revision 23
# speedup vs baseline: 491453.6910x; 1.1154x over previous
# Trainium2 Bass kernel for nn_AttentionBlock (GroupNorm -> QKV -> single-head
# attention over 64x64 tokens -> proj -> residual), B=4, C=256, H=W=64.
#
# Sharding: 8 cores = (batch b in 0..3) x (query-half in {0,1}).  Each core
# receives batch item b's full (C, N=4096) slab, rotated so that its own 2048
# query positions come first.  The program is identical on every core (pure
# SPMD, no collectives); the host slices inputs and reassembles the output.
#
# On-chip layout is channel-major (C on partitions) everywhere except V, which
# is produced directly token-major (n on partitions) so the P@V contraction
# needs no transposes.  Attention is computed as S^T (keys on partitions,
# queries on free axis).  exp() skips max-subtraction: logits here are ~N(0,1)
# (max < ~7), far from fp32 overflow, and softmax is shift-invariant.
#
# The softmax denominator l[q] = sum_n exp(s[n,q]) is a cross-partition sum:
# the 32 exp'd key-tiles are accumulated elementwise on the (otherwise idle)
# GpSimd and Vector engines into two (128,512) partials, and a single fp32
# ones-vector matmul folds the 128 partitions into l.  This keeps the
# TensorEngine (the bottleneck) free of the 128 M=1 matmuls it would otherwise
# spend ~27us on.
#
# All heavy matmuls run in bf16 with fp32 PSUM accumulation; GroupNorm stats
# and the softmax normalization stay fp32.  S-psum / P tiles / q,k drains are
# processed as (128,1024) two-bank tensor ops to halve instruction overhead.

import contextlib

import numpy as np
import ml_dtypes

import concourse.bass as bass
import concourse.bacc as bacc
import concourse.mybir as mybir
import concourse.tile as tile
from concourse.bass_utils import run_bass_kernel_spmd

F32 = mybir.dt.float32
BF16 = mybir.dt.bfloat16

B = 4
C = 256
N = 4096          # tokens per batch item (64*64)
NH = 2048         # tokens per core (query half)
G = 32            # groups
GS = C // G       # channels per group
P = 128
CT = C // P       # 2 channel tiles
NT = N // P       # 32 key tiles
QB = NH // 512    # 4 query blocks of 512
EPS = 1e-6
LOGIT_SCALE = 1.0 / 16.0   # 1/sqrt(C)

TRACE = False
PHASES = ("gn", "qkv", "attn")
LAST_RESULT = None
_CACHED_NC = None


def _build_nc(loop_k=None, fold_qk=True):
    nc = bacc.Bacc()

    x_in = nc.dram_tensor("x_in", [C, N], F32, kind="ExternalInput")
    wqkvT = nc.dram_tensor("wqkvT", [C, 3 * C], BF16, kind="ExternalInput")
    bqkv = nc.dram_tensor("bqkv", [3 * C, 1], F32, kind="ExternalInput")
    bproj = nc.dram_tensor("bproj", [C, 1], F32, kind="ExternalInput")
    gamma_d = nc.dram_tensor("gamma", [C, 1], F32, kind="ExternalInput")
    beta_d = nc.dram_tensor("beta", [C, 1], F32, kind="ExternalInput")
    gsel_d = nc.dram_tensor("gsel", [C, G], F32, kind="ExternalInput")
    gbc_d = nc.dram_tensor("gbc", [G, C], F32, kind="ExternalInput")
    out_d = nc.dram_tensor("out", [C, NH], F32, kind="ExternalOutput")

    with tile.TileContext(nc) as tc:
        with (
            tc.tile_pool(name="persist", bufs=1) as pp,
            tc.tile_pool(name="small", bufs=1) as sp,
            tc.tile_pool(name="ptiles", bufs=4) as ptp,
            tc.tile_pool(name="work", bufs=2) as wkp,
            tc.For_i(0, loop_k, 1) if loop_k else contextlib.nullcontext(),
        ):
            # ---- load inputs -------------------------------------------------
            x_t = []
            for i in range(CT):
                xt = pp.tile([P, N], F32, tag=f"x{i}", name=f"x{i}")
                # split the load so bn_stats can start on early chunks
                for ch in range(4):
                    nc.sync.dma_start(
                        out=xt[:, ch * (N // 4):(ch + 1) * (N // 4)],
                        in_=x_in[i * P:(i + 1) * P,
                                 ch * (N // 4):(ch + 1) * (N // 4)])
                x_t.append(xt)

            wq_t = []
            for i in range(CT):
                wt = pp.tile([P, 3 * C], BF16, tag=f"wqkv{i}", name=f"wq{i}")
                nc.sync.dma_start(out=wt, in_=wqkvT[i * P:(i + 1) * P, :])
                wq_t.append(wt)

            # (768,1) biases -> (128, 6): column j holds rows [128j, 128j+128)
            bq_sb = sp.tile([P, 6], F32, tag="bqkv")
            nc.sync.dma_start(
                out=bq_sb,
                in_=bass.AP(tensor=bqkv, offset=0, ap=[[1, P], [P, 6]]),
            )
            bpj_sb = sp.tile([P, CT], F32, tag="bproj")
            nc.sync.dma_start(
                out=bpj_sb,
                in_=bass.AP(tensor=bproj, offset=0, ap=[[1, P], [P, CT]]),
            )
            gam_sb = sp.tile([P, CT], F32, tag="gamma")
            nc.sync.dma_start(
                out=gam_sb,
                in_=bass.AP(tensor=gamma_d, offset=0, ap=[[1, P], [P, CT]]),
            )
            bet_sb = sp.tile([P, CT], F32, tag="beta")
            nc.sync.dma_start(
                out=bet_sb,
                in_=bass.AP(tensor=beta_d, offset=0, ap=[[1, P], [P, CT]]),
            )
            # fp32 matmuls lower to a single instruction with one sync-wait
            # slot, so their operands must all come from one engine: launder
            # the DMA-loaded selector matrices through a DVE copy.
            gsel_t = []
            for i in range(CT):
                gt0 = sp.tile([P, G], F32, tag=f"gseld{i}", name=f"gt0_{i}")
                nc.sync.dma_start(out=gt0, in_=gsel_d[i * P:(i + 1) * P, :])
                gt = sp.tile([P, G], F32, tag=f"gsel{i}", name=f"gt_{i}")
                nc.vector.tensor_copy(gt, gt0)
                gsel_t.append(gt)
            gbc0 = sp.tile([G, C], F32, tag="gbcd")
            nc.sync.dma_start(out=gbc0, in_=gbc_d[:, :])
            gbc_sb = sp.tile([G, C], F32, tag="gbc")
            nc.vector.tensor_copy(gbc_sb, gbc0)

            ones_f = sp.tile([P, 1], F32, tag="ones_f")
            nc.vector.memset(ones_f, 1.0)
            eps_t = sp.tile([G, 1], F32, tag="eps")
            nc.vector.memset(eps_t, EPS)

            # ---- GroupNorm statistics ---------------------------------------
            # per-channel mean/var via bn_stats (8 subgroups of 512)
            with tc.tile_pool(name="gn_ps", bufs=1, space="PSUM") as gnps:
                stat2 = []
                for i in range(CT):
                    bst = sp.tile([P, 8, 6], F32, tag=f"bnst{i}", name=f"bnst{i}")
                    for s in range(8):
                        nc.vector.bn_stats(
                            out=bst[:, s, :],
                            in_=x_t[i][:, s * 512:(s + 1) * 512],
                        )
                    mv = sp.tile([P, 2], F32, tag=f"mv{i}", name=f"mv{i}")
                    nc.vector.bn_aggr(out=mv, in_=bst)
                    st = sp.tile([P, 2], F32, tag=f"stat2{i}", name=f"st{i}")
                    nc.vector.tensor_copy(st[:, 0:1], mv[:, 0:1])
                    # m2 = var + mean^2
                    nc.vector.tensor_mul(st[:, 1:2], mv[:, 0:1], mv[:, 0:1])
                    nc.vector.tensor_add(st[:, 1:2], st[:, 1:2], mv[:, 1:2])
                    stat2.append(st)

                # group aggregate: (32, 2) = sum_c gsel[c,g]/8 * [mean_c, m2_c]
                ps_g = gnps.tile([G, 2], F32, tag="psg")
                nc.tensor.matmul(ps_g, gsel_t[0], stat2[0], start=True, stop=False)
                nc.tensor.matmul(ps_g, gsel_t[1], stat2[1], start=False, stop=True)

                grp = sp.tile([G, 2], F32, tag="grp")
                nc.vector.tensor_copy(grp, ps_g)
                # var_g = m2_g - mean_g^2 ; rstd = 1/sqrt(var+eps)
                vtmp = sp.tile([G, 1], F32, tag="vtmp")
                nc.vector.tensor_mul(vtmp, grp[:, 0:1], grp[:, 0:1])
                nc.vector.tensor_sub(vtmp, grp[:, 1:2], vtmp)
                srt = sp.tile([G, 1], F32, tag="srt")
                nc.scalar.activation(
                    out=srt, in_=vtmp,
                    func=mybir.ActivationFunctionType.Sqrt,
                    bias=eps_t, scale=1.0,
                )
                mr_g = sp.tile([G, 2], F32, tag="mrg")
                nc.vector.tensor_copy(mr_g[:, 0:1], grp[:, 0:1])
                nc.vector.reciprocal(mr_g[:, 1:2], srt)

                # broadcast back to channels: (128, 2) per c-tile
                scale_c, shift_c = [], []
                for i in range(CT):
                    ps_c = gnps.tile([P, 2], F32, tag="psc", bufs=2, name=f"psc{i}")
                    nc.tensor.matmul(
                        ps_c, gbc_sb[:, i * P:(i + 1) * P], mr_g,
                        start=True, stop=True,
                    )
                    sc = sp.tile([P, 1], F32, tag=f"scale{i}", name=f"sc{i}")
                    sh = sp.tile([P, 1], F32, tag=f"shift{i}", name=f"sh{i}")
                    # scale = rstd * gamma ; shift = beta - mean * scale
                    nc.vector.tensor_mul(sc, ps_c[:, 1:2], gam_sb[:, i:i + 1])
                    nc.vector.tensor_mul(sh, ps_c[:, 0:1], sc)
                    nc.vector.tensor_sub(sh, bet_sb[:, i:i + 1], sh)
                    scale_c.append(sc)
                    shift_c.append(sh)

            # ---- h = GroupNorm(x) in bf16 (ACT); x += bproj in-place (DVE) --
            h_t = []
            for i in range(CT):
                ht = pp.tile([P, N], BF16, tag=f"h{i}", name=f"h{i}")
                if i == 0:
                    nc.scalar.activation(
                        out=ht, in_=x_t[i],
                        func=mybir.ActivationFunctionType.Identity,
                        bias=shift_c[i], scale=scale_c[i],
                    )
                else:
                    nc.vector.tensor_scalar(
                        out=ht, in0=x_t[i],
                        scalar1=scale_c[i], scalar2=shift_c[i],
                        op0=mybir.AluOpType.mult, op1=mybir.AluOpType.add,
                    )
                h_t.append(ht)
            for i in range(CT):
                # x (residual half) + bproj, in place
                nc.vector.tensor_scalar_add(
                    out=x_t[i][:, 0:NH], in0=x_t[i][:, 0:NH],
                    scalar1=bpj_sb[:, i:i + 1],
                )

            # ---- QKV ---------------------------------------------------------
            if fold_qk:
                q_t = h_t          # S consumes h directly
            else:
                q_t = [pp.tile([P, NH], BF16, tag=f"q{i}", name=f"q{i}")
                       for i in range(CT)]
            k_t = [pp.tile([P, N], BF16, tag=f"k{i}", name=f"k{i}")
                   for i in range(CT)]
            v_sb = pp.tile([P, NT, C], BF16, tag="v")

            if "qkv" not in PHASES:
                qps = None
            else:
              with tc.tile_pool(name="qkv_ps", bufs=1, space="PSUM") as qps:
                # With fold_qk (b_qkv q/k parts all zero), S = h^T (Wq^T Wk) h:
                # the host bakes A^T into the k-columns of wqkvT and the S
                # matmuls consume h directly -- no Q computation at all.
                if not fold_qk:
                  for co in range(CT):   # q: only our half, 1024-wide blocks
                    for nb in range(NH // 1024):
                        ps = qps.tile([P, 1024], F32, tag="qk", bufs=2, name="psq")
                        for r in range(2):   # psum bank per matmul group
                            for ci in range(CT):
                                nc.tensor.matmul(
                                    ps[:, r * 512:(r + 1) * 512],
                                    wq_t[ci][:, co * P:(co + 1) * P],
                                    h_t[ci][:, nb * 1024 + r * 512:
                                            nb * 1024 + (r + 1) * 512],
                                    start=(ci == 0), stop=(ci == CT - 1),
                                )
                        if (co + nb) % 2 == 0:
                            nc.scalar.activation(
                                out=q_t[co][:, nb * 1024:(nb + 1) * 1024],
                                in_=ps,
                                func=mybir.ActivationFunctionType.Identity,
                                bias=bq_sb[:, co:co + 1], scale=1.0,
                            )
                        else:
                            nc.vector.tensor_scalar_add(
                                out=q_t[co][:, nb * 1024:(nb + 1) * 1024],
                                in0=ps, scalar1=bq_sb[:, co:co + 1],
                            )
                for co in range(CT):   # k: full token range
                    for nb in range(N // 1024):
                        ps = qps.tile([P, 1024], F32, tag="qk", bufs=2, name="psk")
                        for r in range(2):   # psum bank per matmul group
                            for ci in range(CT):
                                nc.tensor.matmul(
                                    ps[:, r * 512:(r + 1) * 512],
                                    wq_t[ci][:, C + co * P:C + (co + 1) * P],
                                    h_t[ci][:, nb * 1024 + r * 512:
                                            nb * 1024 + (r + 1) * 512],
                                    start=(ci == 0), stop=(ci == CT - 1),
                                )
                        if (co + nb) % 2 == 0:
                            nc.scalar.activation(
                                out=k_t[co][:, nb * 1024:(nb + 1) * 1024],
                                in_=ps,
                                func=mybir.ActivationFunctionType.Identity,
                                bias=bq_sb[:, 2 + co:3 + co], scale=1.0,
                            )
                        else:
                            nc.vector.tensor_scalar_add(
                                out=k_t[co][:, nb * 1024:(nb + 1) * 1024],
                                in0=ps, scalar1=bq_sb[:, 2 + co:3 + co],
                            )
                for i2 in range(NT // 2):   # v: token-major, paired tiles
                    ps = qps.tile([P, 2, C], F32, tag="v", bufs=3, name="psv")
                    for r in range(2):
                        i = 2 * i2 + r
                        for ci in range(CT):
                            nc.tensor.matmul(
                                ps[:, r, :],
                                h_t[ci][:, i * P:(i + 1) * P],
                                wq_t[ci][:, 2 * C:3 * C],
                                start=(ci == 0), stop=(ci == CT - 1),
                            )
                    # v bias is folded into bproj on the host
                    if i2 % 2 == 0:
                        nc.scalar.activation(
                            out=v_sb[:, 2 * i2:2 * i2 + 2, :], in_=ps,
                            func=mybir.ActivationFunctionType.Copy,
                        )
                    else:
                        nc.vector.tensor_copy(
                            v_sb[:, 2 * i2:2 * i2 + 2, :], ps)

            # ---- attention + proj + residual, per query block ----------------
            # The per-block tail (proj, softmax-normalize, residual, store) is
            # emitted AFTER priming the NEXT block's S pipeline, so the PE
            # covers the tail's cross-engine waits with real matmul work.
            with tc.tile_pool(name="att_ps", bufs=1, space="PSUM") as aps:

                def s_mms(i2, qsl):
                    s = aps.tile([P, 2, 512], F32, tag="s", bufs=3,
                                 name="s2")
                    for r in range(2):
                        i = 2 * i2 + r
                        for ci in range(CT):
                            nc.tensor.matmul(
                                s[:, r, :],
                                k_t[ci][:, i * P:(i + 1) * P],
                                q_t[ci][:, qsl],
                                start=(ci == 0), stop=(ci == CT - 1),
                            )
                    return s

                def qb_tail(o01, lac, qsl):
                    # The proj layer is folded into the V weights on the host
                    # (W_pv = w_proj @ W_v), so o01 already holds the
                    # projected, unnormalized output.  Drain it (split
                    # ACT/DVE) to free the psum banks early, then normalize
                    # by 1/l and add the residual.
                    o_sb = wkp.tile([P, 2, 512], BF16, tag="osb", name="osb")
                    nc.scalar.activation(
                        out=o_sb[:, 0, :], in_=o01[:, 0, :],
                        func=mybir.ActivationFunctionType.Copy)
                    nc.vector.tensor_copy(o_sb[:, 1, :], o01[:, 1, :])

                    # fold partitions: l = ones.T @ (lac0 + lac1), then 1/l
                    lps = aps.tile([1, 512], F32, tag="s", bufs=3, name="lps")
                    nc.vector.tensor_add(lac[1], lac[1], lac[0])
                    nc.tensor.matmul(lps, ones_f, lac[1],
                                     start=True, stop=True)
                    recip = wkp.tile([1, 512], F32, tag="recip", name="recip")
                    nc.vector.reciprocal(recip, lps)
                    rbc = wkp.tile([P, 512], F32, tag="rbc", name="rbc")
                    nc.gpsimd.partition_broadcast(rbc, recip)

                    for co in range(CT):
                        f = wkp.tile([P, 512], F32, tag=f"f{co}",
                                     name=f"f{co}")
                        nc.vector.tensor_mul(f, o_sb[:, co, :], rbc)
                        nc.vector.tensor_add(f, f, x_t[co][:, qsl])
                        nc.sync.dma_start(
                            out=out_d[co * P:(co + 1) * P, qsl], in_=f
                        )

                pending = None
                for qb in range(QB):
                    qsl = slice(qb * 512, (qb + 1) * 512)
                    o01 = aps.tile([P, 2, 512], F32, tag="o01", name="o01")
                    lac = [
                        wkp.tile([P, 512], F32, tag="lac0", name="lac0"),
                        wkp.tile([P, 512], F32, tag="lac1", name="lac1"),
                    ]

                    # prime this block's S pipeline (depth 2) ...
                    s_pipe = [s_mms(0, qsl), s_mms(1, qsl)]
                    # ... THEN emit the previous block's tail
                    if pending is not None:
                        qb_tail(*pending)

                    for i2 in range(NT // 2):
                        p2 = ptp.tile([P, 2, 512], BF16, tag="p", name="p2")
                        nc.scalar.activation(
                            out=p2, in_=s_pipe.pop(0),
                            func=mybir.ActivationFunctionType.Exp,
                            bias=0.0, scale=LOGIT_SCALE,
                        )
                        if i2 + 2 < NT // 2:
                            s_pipe.append(s_mms(i2 + 2, qsl))
                        for r in range(2):
                            i = 2 * i2 + r
                            nc.tensor.matmul(
                                o01[:, 0, :], v_sb[:, i, 0:P], p2[:, r, :],
                                start=(i == 0), stop=(i == NT - 1),
                            )
                            nc.tensor.matmul(
                                o01[:, 1, :], v_sb[:, i, P:C], p2[:, r, :],
                                start=(i == 0), stop=(i == NT - 1),
                            )
                        # l partials on Pool / DVE (first update is a copy,
                        # so no memset is needed)
                        if i2 == 0:
                            nc.gpsimd.tensor_copy(lac[0], p2[:, 0, :])
                            nc.vector.tensor_copy(lac[1], p2[:, 1, :])
                        else:
                            nc.gpsimd.tensor_add(lac[0], lac[0], p2[:, 0, :])
                            nc.vector.tensor_add(lac[1], lac[1], p2[:, 1, :])

                    pending = (o01, lac, qsl)
                qb_tail(*pending)
    nc.finalize()
    return nc


def _host_inputs(x, gamma, beta, w_qkv, b_qkv, w_proj, b_proj, fold_qk=True):
    x4 = np.ascontiguousarray(np.asarray(x, np.float32).reshape(B, C, N))
    # proj folds into the V weights: proj(P@V) = P@(V @ w_proj.T), and
    # V = W_v h, so the v-columns of wqkvT become (w_proj @ W_v).T
    wq32 = np.asarray(w_qkv, np.float32)
    wp32 = np.asarray(w_proj, np.float32)
    wqkvT_f = np.ascontiguousarray(wq32.T).copy()
    wqkvT_f[:, 2 * C:3 * C] = (wp32 @ wq32[2 * C:3 * C]).T
    if fold_qk:
        # S = h^T (Wq^T Wk) h: k2 = A h with A = Wq^T Wk; lhsT slice = A^T
        A = wq32[0:C].T @ wq32[C:2 * C]
        wqkvT_f[:, C:2 * C] = A.T
    wqkvT = wqkvT_f.astype(ml_dtypes.bfloat16)
    bqkv = np.ascontiguousarray(np.asarray(b_qkv, np.float32).reshape(3 * C, 1))
    # v-bias is applied on the host side of the algebra:
    # P@(V+b_v)/l = (P@V)/l + b_v, so proj(..)+b_proj gains w_proj @ b_v.
    bproj_eff = (np.asarray(b_proj, np.float32)
                 + np.asarray(w_proj, np.float32) @ np.asarray(
                     b_qkv, np.float32)[2 * C:3 * C])
    bproj = np.ascontiguousarray(bproj_eff.reshape(C, 1))
    gam = np.ascontiguousarray(np.asarray(gamma, np.float32).reshape(C, 1))
    bet = np.ascontiguousarray(np.asarray(beta, np.float32).reshape(C, 1))

    # bn_aggr gives per-channel mean/var over the N positions, so the group
    # combine only averages the GS channels in each group: weight 1/GS.
    gsel = np.zeros((C, G), np.float32)
    gbc = np.zeros((G, C), np.float32)
    for c in range(C):
        gsel[c, c // GS] = 1.0 / GS
        gbc[c // GS, c] = 1.0

    shared = dict(wqkvT=wqkvT, bqkv=bqkv, bproj=bproj,
                  gamma=gam, beta=bet, gsel=gsel, gbc=gbc)
    in_maps = []
    for core in range(8):
        b, half = divmod(core, 2)
        xs = x4[b]
        if half:
            xs = np.concatenate([xs[:, NH:], xs[:, :NH]], axis=1)
        in_maps.append(dict(x_in=np.ascontiguousarray(xs), **shared))
    return in_maps


def kernel(x, gamma, beta, w_qkv, b_qkv, w_proj, b_proj):
    global _CACHED_NC, LAST_RESULT
    # Q is eliminated (S = h^T (Wq^T Wk) h) only when the q/k biases are
    # zero; the k-bias is softmax-invariant regardless, but a nonzero q-bias
    # would need a per-key logit correction, so fall back to the general
    # path in that case.
    fold_qk = not np.any(np.asarray(b_qkv, np.float32)[0:2 * C])
    if _CACHED_NC is None or _CACHED_NC[1] != fold_qk:
        _CACHED_NC = (_build_nc(fold_qk=fold_qk), fold_qk)
    in_maps = _host_inputs(x, gamma, beta, w_qkv, b_qkv, w_proj, b_proj,
                           fold_qk=fold_qk)
    res = run_bass_kernel_spmd(
        _CACHED_NC[0], in_maps, core_ids=list(range(8)), trace=TRACE
    )
    LAST_RESULT = res
    out = np.empty((B, C, N), np.float32)
    for core in range(8):
        b, half = divmod(core, 2)
        out[b][:, half * NH:(half + 1) * NH] = res.results[core]["out"]
    return out.reshape(B, C, 64, 64)
